# revision 1
# baseline (speedup 1.0000x reference)
"""CrossTransformer (depth-3, dim-1024, heads-8, mlp-4096) on 8 TRN2 NeuronCores.

Strategy: pure data-parallel over batch (8 batch elements -> 8 cores, no
collectives). Each core runs the full 3-layer transformer on its own
[1026, 1024] sequence.

Per-core layout:
  - residual c: token-major fp32, 9 partition-tiles of [128, 1024]
  - LN on DVE (bn_stats/bn_aggr + fused tensor_scalar)
  - h transposed to feature-major bf16 via PE transposes (matmul inputs)
  - q,k per head feature-major [hd=128, n]; v token-major with an appended
    ones-column so each attn@v matmul also produces the softmax denominator
    (scores are small -> exp without max subtraction is safe)
  - attention probs ET stored transposed [j, i]; o computed token-major,
    normalized by 1/Z via per-partition tensor_scalar, transposed back
  - FFN: ff1 -> gelu (exact) -> ff2, residual adds from PSUM
  - all matmuls in bf16 (weights pre-cast on host), fp32 accumulation
"""

import numpy as np
import ml_dtypes

import concourse.bass as bass
import concourse.bacc as bacc
import concourse.mybir as mybir
import concourse.tile as tile
from concourse.bass_utils import run_bass_kernel_spmd
from concourse.masks import make_identity

BF16 = mybir.dt.bfloat16
F32 = mybir.dt.float32
AF = mybir.ActivationFunctionType
OP = mybir.AluOpType

N_CORES = 8
DIM = 1024
DEPTH = 3
HEADS = 8
HD = 128
MLP = 4096
EPS = 1e-5
N = 1026                      # tokens = 1 + 1 + 1024
NT = 9                        # token partition tiles
TOK = [128] * 8 + [2]         # valid rows per token tile
NIC = 3
IC = 342                      # free-dim token chunk, 3*342 = 1026
DT = DIM // 128               # 8 feature tiles
MT = MLP // 128               # 32 mlp tiles
SCALE = DIM ** -0.5           # 1/32, note: dim**-0.5 not head_dim**-0.5

_CACHE = {}


def _tok_span(t):
    return t * 128, TOK[t]


def _build():
    nc = bacc.Bacc()
    c0h_d = nc.declare_dram_parameter("c0h", [128, DIM], F32, isOutput=False)
    zz_d = nc.declare_dram_parameter("zz", [DIM, DIM], F32, isOutput=False)
    wqkv_d = nc.declare_dram_parameter("wqkv", [DEPTH, DT, 128, 3 * DIM], BF16, isOutput=False)
    wout_d = nc.declare_dram_parameter("wout", [DEPTH, DT, 128, DIM], BF16, isOutput=False)
    wff1_d = nc.declare_dram_parameter("wff1", [DEPTH, DT, 128, MLP], BF16, isOutput=False)
    wff2_d = nc.declare_dram_parameter("wff2", [DEPTH, MT, 128, DIM], BF16, isOutput=False)
    out_d = nc.declare_dram_parameter("out01", [2, DIM], F32, isOutput=True)

    with tile.TileContext(nc) as tc:
        with (
            tc.tile_pool(name="const", bufs=1) as const,
            tc.tile_pool(name="cpool", bufs=NT) as cpool,
            tc.tile_pool(name="htm", bufs=2) as htm,
            tc.tile_pool(name="hfm", bufs=DT) as hfm,
            tc.tile_pool(name="scratch", bufs=35) as scratch,
            tc.tile_pool(name="vpool", bufs=NT) as vpool,
            tc.tile_pool(name="qkp", bufs=2) as qkp,
            tc.tile_pool(name="ost", bufs=6) as ost,
            tc.tile_pool(name="small", bufs=8) as small,
            tc.tile_pool(name="wsmall", bufs=5) as wsmall,
            tc.tile_pool(name="wbig", bufs=3) as wbig,
            tc.tile_pool(name="psum", bufs=7, space="PSUM") as psum,
        ):
            ident = const.tile([128, 128], BF16, tag="ident")
            make_identity(nc, ident[:])
            eps_t = const.tile([128, 1], F32, tag="eps")
            nc.vector.memset(eps_t[:], EPS)

            # load residual stream c (token-major fp32)
            c_tiles = [cpool.tile([128, DIM], F32, tag="c", name="c") for _ in range(NT)]
            nc.sync.dma_start(out=c_tiles[0][:, :], in_=c0h_d[:, :])
            for t in range(1, NT):
                r0 = 126 + (t - 1) * 128
                nc.sync.dma_start(
                    out=c_tiles[t][: TOK[t], :], in_=zz_d[r0 : r0 + TOK[t], :]
                )

            def layer_norm(h_tiles, need_sink=False):
                """h_tiles: 8 feature-major bf16 [128, N] tiles to fill."""
                for t in range(NT):
                    vt = TOK[t]
                    h_tm = htm.tile([128, DIM], BF16, tag="htm", name="htm")
                    stats = small.tile([128, 2, 6], F32, tag="stats", name="stats")
                    mv = small.tile([128, 2], F32, tag="mv", name="mv")
                    if need_sink:
                        # BNStats' ISA struct has few sync-wait slots; absorb
                        # the DMA producers' waits with a generic DVE op first.
                        sink = small.tile([128, 1], F32, tag="sink", name="sink")
                        nc.vector.tensor_copy(sink[:vt], c_tiles[t][:vt, 0:1])
                    for hf in range(2):
                        nc.vector.bn_stats(
                            stats[:vt, hf, :], c_tiles[t][:vt, hf * 512 : (hf + 1) * 512]
                        )
                    nc.vector.bn_aggr(mv[:vt], stats[:vt])
                    rstd = small.tile([128, 1], F32, tag="rstd", name="rstd")
                    nc.scalar.activation(
                        rstd[:vt], mv[:vt, 1:2], AF.Sqrt, bias=eps_t[:vt]
                    )
                    nc.vector.reciprocal(rstd[:vt], rstd[:vt])
                    # nmr = (mu * -1) * rinv, so the normalize can run on the
                    # Scalar engine as Identity(c * rinv + nmr) (frees DVE,
                    # which otherwise gates the PE transposes)
                    nmr = small.tile([128, 1], F32, tag="nmr", name="nmr")
                    nc.vector.scalar_tensor_tensor(
                        out=nmr[:vt],
                        in0=mv[:vt, 0:1],
                        scalar=-1.0,
                        in1=rstd[:vt],
                        op0=OP.mult,
                        op1=OP.mult,
                    )
                    nc.scalar.activation(
                        h_tm[:vt],
                        c_tiles[t][:vt],
                        AF.Identity,
                        bias=nmr[:vt],
                        scale=rstd[:vt],
                    )
                    t0 = t * 128
                    for dt in range(DT):
                        pst = psum.tile([128, 512], BF16, tag="ps", name="pst")
                        nc.tensor.transpose(
                            pst[:128, :vt], h_tm[:vt, dt * 128 : (dt + 1) * 128], ident[:vt, :vt]
                        )
                        nc.vector.tensor_copy(
                            h_tiles[dt][:, t0 : t0 + vt], pst[:128, :vt]
                        )

            for li in range(DEPTH):
                # ---- LN1 -> h feature-major ----
                h_tiles = [hfm.tile([128, N], BF16, tag="hfm", name="hfm") for _ in range(DT)]
                layer_norm(h_tiles, need_sink=(li == 0))

                # ---- v token-major (+ ones column for the softmax denominator) ----
                v_tiles = [vpool.tile([128, HEADS, HD + 1], BF16, tag="v", name="v") for _ in range(NT)]
                for fc in range(2):
                    wv_sb = wbig.tile([128, DT, 512], BF16, tag="wbig", name="wv")
                    nc.sync.dma_start(
                        out=wv_sb[:],
                        in_=wqkv_d[li, :, :, 2 * DIM + fc * 512 : 2 * DIM + (fc + 1) * 512]
                        .rearrange("a p k -> p a k"),
                    )
                    for jt in range(NT):
                        j0, vj = _tok_span(jt)
                        ps2 = psum.tile([128, 512], F32, tag="ps", name="ps")
                        for dt in range(DT):
                            nc.tensor.matmul(
                                ps2[:vj, :512],
                                h_tiles[dt][:, j0 : j0 + vj],
                                wv_sb[:, dt, :],
                                start=(dt == 0),
                                stop=(dt == DT - 1),
                            )
                        for hh in range(4):
                            nc.scalar.copy(
                                v_tiles[jt][:vj, fc * 4 + hh, 0:HD],
                                ps2[:vj, hh * HD : (hh + 1) * HD],
                            )
                        if fc == 0:
                            nc.vector.memset(v_tiles[jt][:vj, :, HD : HD + 1], 1.0)

                # ---- attention, head by head ----
                o_tiles = [scratch.tile([128, N], BF16, tag="sc", name="sc") for _ in range(HEADS)]
                for h in range(HEADS):
                    q_t = qkp.tile([128, N], BF16, tag="qk", name="qk")
                    k_t = qkp.tile([128, N], BF16, tag="qk", name="qk")
                    wqk_sb = wsmall.tile([128, DT, 2, HD], BF16, tag="wsmall", name="wqk")
                    for qi, base in ((0, h * HD), (1, DIM + h * HD)):
                        nc.sync.dma_start(
                            out=wqk_sb[:, :, qi, :],
                            in_=wqkv_d[li, :, :, base : base + HD].rearrange("a p k -> p a k"),
                        )
                    for qi, dst in ((1, k_t), (0, q_t)):
                        pss = [psum.tile([128, 512], F32, tag="ps", name="ps") for _ in range(NIC)]
                        for dt in range(DT):
                            for ic in range(NIC):
                                nc.tensor.matmul(
                                    pss[ic][:, :IC],
                                    wqk_sb[:, dt, qi, :],
                                    h_tiles[dt][:, ic * IC : (ic + 1) * IC],
                                    start=(dt == 0),
                                    stop=(dt == DT - 1),
                                )
                        for ic in range(NIC):
                            nc.scalar.copy(dst[:, ic * IC : (ic + 1) * IC], pss[ic][:, :IC])

                    # scores^T [j, i] -> exp
                    et = [scratch.tile([128, N], BF16, tag="sc", name="sc") for _ in range(NT)]
                    for jt in range(NT):
                        j0, vj = _tok_span(jt)
                        for ic in range(NIC):
                            ps = psum.tile([128, 512], F32, tag="ps", name="ps")
                            nc.tensor.matmul(
                                ps[:vj, :IC],
                                k_t[:, j0 : j0 + vj],
                                q_t[:, ic * IC : (ic + 1) * IC],
                                start=True,
                                stop=True,
                            )
                            nc.scalar.activation(
                                et[jt][:vj, ic * IC : (ic + 1) * IC],
                                ps[:vj, :IC],
                                AF.Exp,
                                scale=SCALE,
                            )

                    # o = attn @ v, token-major, with fused denominator column.
                    # Lag the (DVE-gated) transpose one iteration behind the
                    # po matmuls so the in-order PE stream never stalls on DVE.
                    def emit_o_tail(o_st, i0, vi, h=h):
                        pt = psum.tile([128, 512], BF16, tag="ps", name="pt")
                        nc.tensor.transpose(pt[:HD, :vi], o_st[:vi, :HD], ident[:vi, :vi])
                        nc.scalar.copy(o_tiles[h][:, i0 : i0 + vi], pt[:HD, :vi])

                    pending = []
                    for it in range(NT):
                        i0, vi = _tok_span(it)
                        po = psum.tile([128, 512], F32, tag="ps", name="ps")
                        for jt in range(NT):
                            j0, vj = _tok_span(jt)
                            nc.tensor.matmul(
                                po[:vi, 0 : HD + 1],
                                et[jt][:vj, i0 : i0 + vi],
                                v_tiles[jt][:vj, h, :],
                                start=(jt == 0),
                                stop=(jt == NT - 1),
                            )
                        zi = small.tile([128, 1], F32, tag="zi", name="zi")
                        nc.vector.reciprocal(zi[:vi], po[:vi, HD : HD + 1])
                        o_st = ost.tile([128, HD], BF16, tag="ost", name="ost")
                        nc.vector.tensor_scalar_mul(o_st[:vi], po[:vi, 0:HD], zi[:vi])
                        if len(pending) >= 4:
                            emit_o_tail(*pending.pop(0))
                        pending.append((o_st, i0, vi))
                    for p in pending:
                        emit_o_tail(*p)
                    pending = []

                # ---- out projection + residual ----
                for fc in range(2):
                    wo_sb = wbig.tile([128, DT, 512], BF16, tag="wbig", name="wo")
                    nc.sync.dma_start(
                        out=wo_sb[:],
                        in_=wout_d[li, :, :, fc * 512 : (fc + 1) * 512].rearrange("a p k -> p a k"),
                    )
                    for it in range(NT):
                        i0, vi = _tok_span(it)
                        pp = psum.tile([128, 512], F32, tag="ps", name="ps")
                        for dt in range(DT):
                            nc.tensor.matmul(
                                pp[:vi, :512],
                                o_tiles[dt][:, i0 : i0 + vi],
                                wo_sb[:, dt, :],
                                start=(dt == 0),
                                stop=(dt == DT - 1),
                            )
                        nc.vector.tensor_add(
                            c_tiles[it][:vi, fc * 512 : (fc + 1) * 512],
                            c_tiles[it][:vi, fc * 512 : (fc + 1) * 512],
                            pp[:vi, :512],
                        )

                # ---- LN2 -> h2 feature-major ----
                h2_tiles = [hfm.tile([128, N], BF16, tag="hfm", name="hfm") for _ in range(DT)]
                layer_norm(h2_tiles)

                # ---- ff1 + exact gelu ----
                g_tiles = [scratch.tile([128, N], BF16, tag="sc", name="sc") for _ in range(MT)]
                for mc in range(MT):
                    w1_sb = wsmall.tile([128, DT, HD], BF16, tag="wsmall", name="w1")
                    nc.sync.dma_start(
                        out=w1_sb[:],
                        in_=wff1_d[li, :, :, mc * HD : (mc + 1) * HD].rearrange("a p k -> p a k"),
                    )
                    pg = [psum.tile([128, 512], F32, tag="ps", name="ps") for _ in range(NIC)]
                    for dt in range(DT):
                        for ic in range(NIC):
                            nc.tensor.matmul(
                                pg[ic][:, :IC],
                                w1_sb[:, dt, :],
                                h2_tiles[dt][:, ic * IC : (ic + 1) * IC],
                                start=(dt == 0),
                                stop=(dt == DT - 1),
                            )
                    for ic in range(NIC):
                        nc.scalar.activation(
                            g_tiles[mc][:, ic * IC : (ic + 1) * IC],
                            pg[ic][:, :IC],
                            AF.Gelu,
                        )

                # ---- ff2 + residual ----
                for ig in range(3):
                    pf = {}
                    for itg in range(3):
                        for fc in range(2):
                            pf[itg, fc] = psum.tile([128, 512], F32, tag="ps", name="pf")
                    for wc in range(MT // 4):
                        w2_sb = wbig.tile([128, 4, DIM], BF16, tag="wbig", name="w2")
                        nc.sync.dma_start(
                            out=w2_sb[:],
                            in_=wff2_d[li, 4 * wc : 4 * wc + 4].rearrange("a p k -> p a k"),
                        )
                        for mi in range(4):
                            mt = 4 * wc + mi
                            for itg in range(3):
                                it = 3 * ig + itg
                                i0, vi = _tok_span(it)
                                for fc in range(2):
                                    nc.tensor.matmul(
                                        pf[itg, fc][:vi, :512],
                                        g_tiles[mt][:, i0 : i0 + vi],
                                        w2_sb[:, mi, fc * 512 : (fc + 1) * 512],
                                        start=(mt == 0),
                                        stop=(mt == MT - 1),
                                    )
                    for itg in range(3):
                        it = 3 * ig + itg
                        i0, vi = _tok_span(it)
                        for fc in range(2):
                            nc.vector.tensor_add(
                                c_tiles[it][:vi, fc * 512 : (fc + 1) * 512],
                                c_tiles[it][:vi, fc * 512 : (fc + 1) * 512],
                                pf[itg, fc][:vi, :512],
                            )

            nc.sync.dma_start(out=out_d[:, :], in_=c_tiles[0][0:2, :])

    nc.finalize()
    return nc


def _prep_inputs(inputs):
    bf = ml_dtypes.bfloat16
    wqkv = np.ascontiguousarray(
        np.asarray(inputs["qkv_w"], dtype=np.float32).reshape(DEPTH, DT, 128, 3 * DIM)
    ).astype(bf)
    wout = np.ascontiguousarray(
        np.asarray(inputs["out_w"], dtype=np.float32).reshape(DEPTH, DT, 128, DIM)
    ).astype(bf)
    wff1 = np.ascontiguousarray(
        np.asarray(inputs["ff1_w"], dtype=np.float32).reshape(DEPTH, DT, 128, MLP)
    ).astype(bf)
    wff2 = np.ascontiguousarray(
        np.asarray(inputs["ff2_w"], dtype=np.float32).reshape(DEPTH, MT, 128, DIM)
    ).astype(bf)
    x = np.asarray(inputs["x"], dtype=np.float32)
    z = np.asarray(inputs["z"], dtype=np.float32)
    zz = np.asarray(inputs["zz"], dtype=np.float32)
    in_maps = []
    for b in range(N_CORES):
        in_maps.append(
            {
                "c0h": np.ascontiguousarray(
                    np.concatenate([x[b], z[b], zz[b][:126]], axis=0)
                ),
                "zz": np.ascontiguousarray(zz[b]),
                "wqkv": wqkv,
                "wout": wout,
                "wff1": wff1,
                "wff2": wff2,
            }
        )
    return in_maps


def kernel(**inputs):
    if "nc" not in _CACHE:
        _CACHE["nc"] = _build()
    nc = _CACHE["nc"]
    in_maps = _prep_inputs(inputs)
    res = run_bass_kernel_spmd(nc, in_maps, core_ids=list(range(N_CORES)))
    out1 = np.stack([res.results[b]["out01"][0:1, :] for b in range(N_CORES)])
    out2 = np.stack([res.results[b]["out01"][1:2, :] for b in range(N_CORES)])
    return out1.astype(np.float32), out2.astype(np.float32)



# revision 8
# speedup vs baseline: 1.5290x; 1.5290x over previous
"""CrossTransformer (depth-3, dim-1024, heads-8, mlp-4096) on 8 TRN2 NeuronCores.

Strategy: pure data-parallel over batch (8 batch elements -> 8 cores, no
collectives). Each core runs the full 3-layer transformer on its own
[1026, 1024] sequence.

Per-core layout:
  - residual c: token-major fp32, 9 partition-tiles of [128, 1024]
  - LN on DVE (bn_stats/bn_aggr + fused tensor_scalar)
  - h transposed to feature-major bf16 via PE transposes (matmul inputs)
  - q,k per head feature-major [hd=128, n]; v token-major with an appended
    ones-column so each attn@v matmul also produces the softmax denominator
    (scores are small -> exp without max subtraction is safe)
  - attention probs ET stored transposed [j, i]; o computed token-major,
    normalized by 1/Z via per-partition tensor_scalar, transposed back
  - FFN: ff1 -> gelu (exact) -> ff2, residual adds from PSUM
  - all matmuls in bf16 (weights pre-cast on host), fp32 accumulation
"""

import numpy as np
import ml_dtypes

import concourse.bass as bass
import concourse.bacc as bacc
import concourse.mybir as mybir
import concourse.tile as tile
from concourse.bass_utils import run_bass_kernel_spmd
from concourse.masks import make_identity

BF16 = mybir.dt.bfloat16
F32 = mybir.dt.float32
AF = mybir.ActivationFunctionType
OP = mybir.AluOpType

N_CORES = 8
DIM = 1024
DEPTH = 3
HEADS = 8
HD = 128
MLP = 4096
EPS = 1e-5
N = 1026                      # tokens = 1 + 1 + 1024
NT = 9                        # token partition tiles
TOK = [128] * 8 + [2]         # valid rows per token tile
NIC = 3
IC = 342                      # free-dim token chunk, 3*342 = 1026
DT = DIM // 128               # 8 feature tiles
MT = MLP // 128               # 32 mlp tiles
SCALE = DIM ** -0.5           # 1/32, note: dim**-0.5 not head_dim**-0.5

_CACHE = {}


def _tok_span(t):
    return t * 128, TOK[t]


def _build():
    nc = bacc.Bacc()
    c0h_d = nc.declare_dram_parameter("c0h", [128, DIM], F32, isOutput=False)
    zz_d = nc.declare_dram_parameter("zz", [DIM, DIM], F32, isOutput=False)
    wqkv_d = nc.declare_dram_parameter("wqkv", [DEPTH, DT, 128, 3 * DIM], BF16, isOutput=False)
    wout_d = nc.declare_dram_parameter("wout", [DEPTH, DT, 128, DIM], BF16, isOutput=False)
    wff1_d = nc.declare_dram_parameter("wff1", [DEPTH, DT, 128, MLP], BF16, isOutput=False)
    wff2_d = nc.declare_dram_parameter("wff2", [DEPTH, MT, 128, DIM], BF16, isOutput=False)
    out_d = nc.declare_dram_parameter("out01", [2, DIM], F32, isOutput=True)

    with tile.TileContext(nc) as tc:
        with (
            tc.tile_pool(name="const", bufs=1) as const,
            tc.tile_pool(name="cpool", bufs=NT) as cpool,
            tc.tile_pool(name="htm", bufs=2) as htm,
            tc.tile_pool(name="hfm", bufs=DT) as hfm,
            tc.tile_pool(name="scratch", bufs=35) as scratch,
            tc.tile_pool(name="vpool", bufs=NT) as vpool,
            tc.tile_pool(name="qkp", bufs=2) as qkp,
            tc.tile_pool(name="ost", bufs=6) as ost,
            tc.tile_pool(name="small", bufs=8) as small,
            tc.tile_pool(name="wsmall", bufs=5) as wsmall,
            tc.tile_pool(name="wbig", bufs=3) as wbig,
            tc.tile_pool(name="psum", bufs=7, space="PSUM") as psum,
        ):
            ident = const.tile([128, 128], BF16, tag="ident")
            make_identity(nc, ident[:])
            eps_t = const.tile([128, 1], F32, tag="eps")
            nc.vector.memset(eps_t[:], EPS)

            # load residual stream c (token-major fp32)
            c_tiles = [cpool.tile([128, DIM], F32, tag="c", name="c") for _ in range(NT)]
            nc.sync.dma_start(out=c_tiles[0][:, :], in_=c0h_d[:, :])
            for t in range(1, NT):
                r0 = 126 + (t - 1) * 128
                nc.sync.dma_start(
                    out=c_tiles[t][: TOK[t], :], in_=zz_d[r0 : r0 + TOK[t], :]
                )

            def layer_norm(h_tiles, need_sink=False):
                """h_tiles: 8 feature-major bf16 [128, N] tiles to fill."""
                for t in range(NT):
                    vt = TOK[t]
                    h_tm = htm.tile([128, DIM], BF16, tag="htm", name="htm")
                    stats = small.tile([128, 2, 6], F32, tag="stats", name="stats")
                    mv = small.tile([128, 2], F32, tag="mv", name="mv")
                    if need_sink:
                        # BNStats' ISA struct has few sync-wait slots; absorb
                        # the DMA producers' waits with a generic DVE op first.
                        sink = small.tile([128, 1], F32, tag="sink", name="sink")
                        nc.vector.tensor_copy(sink[:vt], c_tiles[t][:vt, 0:1])
                    for hf in range(2):
                        nc.vector.bn_stats(
                            stats[:vt, hf, :], c_tiles[t][:vt, hf * 512 : (hf + 1) * 512]
                        )
                    nc.vector.bn_aggr(mv[:vt], stats[:vt])
                    rstd = small.tile([128, 1], F32, tag="rstd", name="rstd")
                    nc.scalar.activation(
                        rstd[:vt], mv[:vt, 1:2], AF.Sqrt, bias=eps_t[:vt]
                    )
                    nc.vector.reciprocal(rstd[:vt], rstd[:vt])
                    # nmr = (mu * -1) * rinv, so the normalize can run on the
                    # Scalar engine as Identity(c * rinv + nmr) (frees DVE,
                    # which otherwise gates the PE transposes)
                    nmr = small.tile([128, 1], F32, tag="nmr", name="nmr")
                    nc.vector.scalar_tensor_tensor(
                        out=nmr[:vt],
                        in0=mv[:vt, 0:1],
                        scalar=-1.0,
                        in1=rstd[:vt],
                        op0=OP.mult,
                        op1=OP.mult,
                    )
                    nc.scalar.activation(
                        h_tm[:vt],
                        c_tiles[t][:vt],
                        AF.Identity,
                        bias=nmr[:vt],
                        scale=rstd[:vt],
                    )
                    t0 = t * 128
                    for dt in range(DT):
                        pst = psum.tile([128, 512], BF16, tag="ps", name="pst")
                        nc.tensor.transpose(
                            pst[:128, :vt], h_tm[:vt, dt * 128 : (dt + 1) * 128], ident[:vt, :vt]
                        )
                        nc.vector.tensor_copy(
                            h_tiles[dt][:, t0 : t0 + vt], pst[:128, :vt]
                        )

            for li in range(DEPTH):
                last = li == DEPTH - 1
                # ---- LN1 -> h feature-major ----
                h_tiles = [hfm.tile([128, N], BF16, tag="hfm", name="hfm") for _ in range(DT)]
                layer_norm(h_tiles, need_sink=(li == 0))

                # ---- v token-major (+ ones column for the softmax denominator) ----
                v_tiles = [vpool.tile([128, HEADS, HD + 1], BF16, tag="v", name="v") for _ in range(NT)]
                for fc in range(2):
                    wv_sb = wbig.tile([128, DT, 512], BF16, tag="wbig", name="wv")
                    nc.sync.dma_start(
                        out=wv_sb[:],
                        in_=wqkv_d[li, :, :, 2 * DIM + fc * 512 : 2 * DIM + (fc + 1) * 512]
                        .rearrange("a p k -> p a k"),
                    )
                    for jt in range(NT):
                        j0, vj = _tok_span(jt)
                        ps2 = psum.tile([128, 512], F32, tag="ps", name="ps")
                        for dt in range(DT):
                            nc.tensor.matmul(
                                ps2[:vj, :512],
                                h_tiles[dt][:, j0 : j0 + vj],
                                wv_sb[:, dt, :],
                                start=(dt == 0),
                                stop=(dt == DT - 1),
                            )
                        for hh in range(4):
                            nc.scalar.copy(
                                v_tiles[jt][:vj, fc * 4 + hh, 0:HD],
                                ps2[:vj, hh * HD : (hh + 1) * HD],
                            )
                        if fc == 0:
                            nc.vector.memset(v_tiles[jt][:vj, :, HD : HD + 1], 1.0)

                # ---- attention, head by head ----
                o_tiles = None if last else [
                    scratch.tile([128, N], BF16, tag="sc", name="sc") for _ in range(HEADS)
                ]
                o01f = [
                    small.tile([128, 2], BF16, tag="o01f", name="o01f", bufs=HEADS)
                    for _ in range(HEADS)
                ] if last else None
                for h in range(HEADS):
                    q_t = qkp.tile([128, N], BF16, tag="qk", name="qk")
                    k_t = qkp.tile([128, N], BF16, tag="qk", name="qk")
                    wqk_sb = wsmall.tile([128, DT, 2, HD], BF16, tag="wsmall", name="wqk")
                    for qi, base in ((0, h * HD), (1, DIM + h * HD)):
                        nc.sync.dma_start(
                            out=wqk_sb[:, :, qi, :],
                            in_=wqkv_d[li, :, :, base : base + HD].rearrange("a p k -> p a k"),
                        )
                    if last:
                        # k for all tokens; q only for tokens 0-1
                        pss = [psum.tile([128, 512], F32, tag="ps", name="ps") for _ in range(NIC)]
                        for dt in range(DT):
                            for ic in range(NIC):
                                nc.tensor.matmul(
                                    pss[ic][:, :IC],
                                    wqk_sb[:, dt, 1, :],
                                    h_tiles[dt][:, ic * IC : (ic + 1) * IC],
                                    start=(dt == 0),
                                    stop=(dt == DT - 1),
                                )
                        for ic in range(NIC):
                            nc.scalar.copy(k_t[:, ic * IC : (ic + 1) * IC], pss[ic][:, :IC])
                        q_ps = psum.tile([128, 512], F32, tag="ps", name="ps")
                        for dt in range(DT):
                            nc.tensor.matmul(
                                q_ps[:, 0:2],
                                wqk_sb[:, dt, 0, :],
                                h_tiles[dt][:, 0:2],
                                start=(dt == 0),
                                stop=(dt == DT - 1),
                            )
                        nc.scalar.copy(q_t[:, 0:2], q_ps[:, 0:2])

                        # scores for 2 query tokens, transposed [j, i] -> exp
                        et01 = [
                            small.tile([128, 2], BF16, tag="et01", name="et01", bufs=NT)
                            for _ in range(NT)
                        ]
                        for jt in range(NT):
                            j0, vj = _tok_span(jt)
                            ps = psum.tile([128, 512], F32, tag="ps", name="ps")
                            nc.tensor.matmul(
                                ps[:vj, 0:2],
                                k_t[:, j0 : j0 + vj],
                                q_t[:, 0:2],
                                start=True,
                                stop=True,
                            )
                            nc.scalar.activation(
                                et01[jt][:vj, 0:2], ps[:vj, 0:2], AF.Exp, scale=SCALE
                            )

                        # o for 2 tokens (ones column gives the denominator)
                        po = psum.tile([128, 512], F32, tag="ps", name="ps")
                        for jt in range(NT):
                            j0, vj = _tok_span(jt)
                            nc.tensor.matmul(
                                po[:2, 0 : HD + 1],
                                et01[jt][:vj, 0:2],
                                v_tiles[jt][:vj, h, :],
                                start=(jt == 0),
                                stop=(jt == NT - 1),
                            )
                        zi = small.tile([128, 1], F32, tag="zi", name="zi")
                        nc.vector.reciprocal(zi[:2], po[:2, HD : HD + 1])
                        o_st = ost.tile([128, HD], BF16, tag="ost", name="ost")
                        nc.vector.tensor_scalar_mul(o_st[:2], po[:2, 0:HD], zi[:2])
                        pt = psum.tile([128, 512], BF16, tag="ps", name="pt")
                        nc.tensor.transpose(pt[:HD, :2], o_st[:2, :HD], ident[:2, :2])
                        nc.scalar.copy(o01f[h][:, 0:2], pt[:HD, :2])
                        continue
                    for qi, dst in ((1, k_t), (0, q_t)):
                        pss = [psum.tile([128, 512], F32, tag="ps", name="ps") for _ in range(NIC)]
                        for dt in range(DT):
                            for ic in range(NIC):
                                nc.tensor.matmul(
                                    pss[ic][:, :IC],
                                    wqk_sb[:, dt, qi, :],
                                    h_tiles[dt][:, ic * IC : (ic + 1) * IC],
                                    start=(dt == 0),
                                    stop=(dt == DT - 1),
                                )
                        for ic in range(NIC):
                            nc.scalar.copy(dst[:, ic * IC : (ic + 1) * IC], pss[ic][:, :IC])

                    # scores^T [j, i] -> exp
                    et = [scratch.tile([128, N], BF16, tag="sc", name="sc") for _ in range(NT)]
                    for jt in range(NT):
                        j0, vj = _tok_span(jt)
                        for ic in range(NIC):
                            ps = psum.tile([128, 512], F32, tag="ps", name="ps")
                            nc.tensor.matmul(
                                ps[:vj, :IC],
                                k_t[:, j0 : j0 + vj],
                                q_t[:, ic * IC : (ic + 1) * IC],
                                start=True,
                                stop=True,
                            )
                            nc.scalar.activation(
                                et[jt][:vj, ic * IC : (ic + 1) * IC],
                                ps[:vj, :IC],
                                AF.Exp,
                                scale=SCALE,
                            )

                    # o = attn @ v, token-major, with fused denominator column.
                    # Lag the (DVE-gated) transpose one iteration behind the
                    # po matmuls so the in-order PE stream never stalls on DVE.
                    def emit_o_tail(o_st, i0, vi, h=h):
                        pt = psum.tile([128, 512], BF16, tag="ps", name="pt")
                        nc.tensor.transpose(pt[:HD, :vi], o_st[:vi, :HD], ident[:vi, :vi])
                        nc.scalar.copy(o_tiles[h][:, i0 : i0 + vi], pt[:HD, :vi])

                    pending = []
                    for it in range(NT):
                        i0, vi = _tok_span(it)
                        po = psum.tile([128, 512], F32, tag="ps", name="ps")
                        for jt in range(NT):
                            j0, vj = _tok_span(jt)
                            nc.tensor.matmul(
                                po[:vi, 0 : HD + 1],
                                et[jt][:vj, i0 : i0 + vi],
                                v_tiles[jt][:vj, h, :],
                                start=(jt == 0),
                                stop=(jt == NT - 1),
                            )
                        zi = small.tile([128, 1], F32, tag="zi", name="zi")
                        nc.vector.reciprocal(zi[:vi], po[:vi, HD : HD + 1])
                        o_st = ost.tile([128, HD], BF16, tag="ost", name="ost")
                        nc.vector.tensor_scalar_mul(o_st[:vi], po[:vi, 0:HD], zi[:vi])
                        if len(pending) >= 4:
                            emit_o_tail(*pending.pop(0))
                        pending.append((o_st, i0, vi))
                    for p in pending:
                        emit_o_tail(*p)
                    pending = []

                # ---- out projection + residual ----
                for fc in range(2):
                    wo_sb = wbig.tile([128, DT, 512], BF16, tag="wbig", name="wo")
                    nc.sync.dma_start(
                        out=wo_sb[:],
                        in_=wout_d[li, :, :, fc * 512 : (fc + 1) * 512].rearrange("a p k -> p a k"),
                    )
                    for it in range(1 if last else NT):
                        i0, vi = (0, 2) if last else _tok_span(it)
                        pp = psum.tile([128, 512], F32, tag="ps", name="ps")
                        for dt in range(DT):
                            nc.tensor.matmul(
                                pp[:vi, :512],
                                o01f[dt][:, 0:2] if last else o_tiles[dt][:, i0 : i0 + vi],
                                wo_sb[:, dt, :],
                                start=(dt == 0),
                                stop=(dt == DT - 1),
                            )
                        nc.vector.tensor_add(
                            c_tiles[it][:vi, fc * 512 : (fc + 1) * 512],
                            c_tiles[it][:vi, fc * 512 : (fc + 1) * 512],
                            pp[:vi, :512],
                        )

                if last:
                    # ---- LN2 + FFN on tokens 0-1 only ----
                    stats = small.tile([128, 2, 6], F32, tag="stats", name="stats")
                    mv = small.tile([128, 2], F32, tag="mv", name="mv")
                    for hf in range(2):
                        nc.vector.bn_stats(
                            stats[:2, hf, :], c_tiles[0][:2, hf * 512 : (hf + 1) * 512]
                        )
                    nc.vector.bn_aggr(mv[:2], stats[:2])
                    rstd = small.tile([128, 1], F32, tag="rstd", name="rstd")
                    nc.scalar.activation(rstd[:2], mv[:2, 1:2], AF.Sqrt, bias=eps_t[:2])
                    nc.vector.reciprocal(rstd[:2], rstd[:2])
                    nmr = small.tile([128, 1], F32, tag="nmr", name="nmr")
                    nc.vector.scalar_tensor_tensor(
                        out=nmr[:2],
                        in0=mv[:2, 0:1],
                        scalar=-1.0,
                        in1=rstd[:2],
                        op0=OP.mult,
                        op1=OP.mult,
                    )
                    h2tm = htm.tile([128, DIM], BF16, tag="htm", name="h2tm")
                    nc.scalar.activation(
                        h2tm[:2], c_tiles[0][:2], AF.Identity, bias=nmr[:2], scale=rstd[:2]
                    )
                    h2f = [
                        small.tile([128, 2], BF16, tag="h2f", name="h2f", bufs=DT)
                        for _ in range(DT)
                    ]
                    for dt in range(DT):
                        pst = psum.tile([128, 512], BF16, tag="ps", name="pst")
                        nc.tensor.transpose(
                            pst[:128, :2], h2tm[:2, dt * 128 : (dt + 1) * 128], ident[:2, :2]
                        )
                        nc.vector.tensor_copy(h2f[dt][:, 0:2], pst[:128, :2])

                    # ff1 (2 tokens): h2f stationary, stream w1 in 512-col chunks
                    g01 = scratch.tile([128, MLP], BF16, tag="g01", name="g01", bufs=1)
                    for ch in range(MLP // 512):
                        w1_sb = wbig.tile([128, DT, 512], BF16, tag="wbig", name="w1b")
                        nc.sync.dma_start(
                            out=w1_sb[:],
                            in_=wff1_d[li, :, :, ch * 512 : (ch + 1) * 512]
                            .rearrange("a p k -> p a k"),
                        )
                        pg01 = psum.tile([128, 512], F32, tag="ps", name="pg01")
                        for dt in range(DT):
                            nc.tensor.matmul(
                                pg01[:2, :512],
                                h2f[dt][:, 0:2],
                                w1_sb[:, dt, :],
                                start=(dt == 0),
                                stop=(dt == DT - 1),
                            )
                        nc.scalar.activation(
                            g01[:2, ch * 512 : (ch + 1) * 512], pg01[:2, :512], AF.Gelu
                        )

                    # transpose g01 -> feature-major per mlp tile
                    g01f = [
                        small.tile([128, 2], BF16, tag="g01f", name="g01f", bufs=MT)
                        for _ in range(MT)
                    ]
                    for mt in range(MT):
                        pst = psum.tile([128, 512], BF16, tag="ps", name="pst")
                        nc.tensor.transpose(
                            pst[:128, :2], g01[:2, mt * 128 : (mt + 1) * 128], ident[:2, :2]
                        )
                        nc.vector.tensor_copy(g01f[mt][:, 0:2], pst[:128, :2])

                    # ff2 (2 tokens) + residual
                    pf2 = [psum.tile([128, 512], F32, tag="ps", name="pf2") for _ in range(2)]
                    for wc in range(MT // 4):
                        w2_sb = wbig.tile([128, 4, DIM], BF16, tag="wbig", name="w2")
                        nc.sync.dma_start(
                            out=w2_sb[:],
                            in_=wff2_d[li, 4 * wc : 4 * wc + 4].rearrange("a p k -> p a k"),
                        )
                        for mi in range(4):
                            mt = 4 * wc + mi
                            for fc in range(2):
                                nc.tensor.matmul(
                                    pf2[fc][:2, :512],
                                    g01f[mt][:, 0:2],
                                    w2_sb[:, mi, fc * 512 : (fc + 1) * 512],
                                    start=(mt == 0),
                                    stop=(mt == MT - 1),
                                )
                    for fc in range(2):
                        nc.vector.tensor_add(
                            c_tiles[0][:2, fc * 512 : (fc + 1) * 512],
                            c_tiles[0][:2, fc * 512 : (fc + 1) * 512],
                            pf2[fc][:2, :512],
                        )
                    continue

                # ---- LN2 -> h2 feature-major ----
                h2_tiles = [hfm.tile([128, N], BF16, tag="hfm", name="hfm") for _ in range(DT)]
                layer_norm(h2_tiles)

                # ---- ff1 + exact gelu ----
                g_tiles = [scratch.tile([128, N], BF16, tag="sc", name="sc") for _ in range(MT)]
                for mc in range(MT):
                    w1_sb = wsmall.tile([128, DT, HD], BF16, tag="wsmall", name="w1")
                    nc.sync.dma_start(
                        out=w1_sb[:],
                        in_=wff1_d[li, :, :, mc * HD : (mc + 1) * HD].rearrange("a p k -> p a k"),
                    )
                    pg = [psum.tile([128, 512], F32, tag="ps", name="ps") for _ in range(NIC)]
                    for dt in range(DT):
                        for ic in range(NIC):
                            nc.tensor.matmul(
                                pg[ic][:, :IC],
                                w1_sb[:, dt, :],
                                h2_tiles[dt][:, ic * IC : (ic + 1) * IC],
                                start=(dt == 0),
                                stop=(dt == DT - 1),
                            )
                    for ic in range(NIC):
                        nc.scalar.activation(
                            g_tiles[mc][:, ic * IC : (ic + 1) * IC],
                            pg[ic][:, :IC],
                            AF.Gelu,
                        )

                # ---- ff2 + residual ----
                for ig in range(3):
                    pf = {}
                    for itg in range(3):
                        for fc in range(2):
                            pf[itg, fc] = psum.tile([128, 512], F32, tag="ps", name="pf")
                    for wc in range(MT // 4):
                        w2_sb = wbig.tile([128, 4, DIM], BF16, tag="wbig", name="w2")
                        nc.sync.dma_start(
                            out=w2_sb[:],
                            in_=wff2_d[li, 4 * wc : 4 * wc + 4].rearrange("a p k -> p a k"),
                        )
                        for mi in range(4):
                            mt = 4 * wc + mi
                            for itg in range(3):
                                it = 3 * ig + itg
                                i0, vi = _tok_span(it)
                                for fc in range(2):
                                    nc.tensor.matmul(
                                        pf[itg, fc][:vi, :512],
                                        g_tiles[mt][:, i0 : i0 + vi],
                                        w2_sb[:, mi, fc * 512 : (fc + 1) * 512],
                                        start=(mt == 0),
                                        stop=(mt == MT - 1),
                                    )
                    for itg in range(3):
                        it = 3 * ig + itg
                        i0, vi = _tok_span(it)
                        for fc in range(2):
                            nc.vector.tensor_add(
                                c_tiles[it][:vi, fc * 512 : (fc + 1) * 512],
                                c_tiles[it][:vi, fc * 512 : (fc + 1) * 512],
                                pf[itg, fc][:vi, :512],
                            )

            nc.sync.dma_start(out=out_d[:, :], in_=c_tiles[0][0:2, :])

    nc.finalize()
    return nc


def _prep_inputs(inputs):
    bf = ml_dtypes.bfloat16
    wqkv = np.ascontiguousarray(
        np.asarray(inputs["qkv_w"], dtype=np.float32).reshape(DEPTH, DT, 128, 3 * DIM)
    ).astype(bf)
    wout = np.ascontiguousarray(
        np.asarray(inputs["out_w"], dtype=np.float32).reshape(DEPTH, DT, 128, DIM)
    ).astype(bf)
    wff1 = np.ascontiguousarray(
        np.asarray(inputs["ff1_w"], dtype=np.float32).reshape(DEPTH, DT, 128, MLP)
    ).astype(bf)
    wff2 = np.ascontiguousarray(
        np.asarray(inputs["ff2_w"], dtype=np.float32).reshape(DEPTH, MT, 128, DIM)
    ).astype(bf)
    x = np.asarray(inputs["x"], dtype=np.float32)
    z = np.asarray(inputs["z"], dtype=np.float32)
    zz = np.asarray(inputs["zz"], dtype=np.float32)
    in_maps = []
    for b in range(N_CORES):
        in_maps.append(
            {
                "c0h": np.ascontiguousarray(
                    np.concatenate([x[b], z[b], zz[b][:126]], axis=0)
                ),
                "zz": np.ascontiguousarray(zz[b]),
                "wqkv": wqkv,
                "wout": wout,
                "wff1": wff1,
                "wff2": wff2,
            }
        )
    return in_maps


def kernel(**inputs):
    if "nc" not in _CACHE:
        _CACHE["nc"] = _build()
    nc = _CACHE["nc"]
    in_maps = _prep_inputs(inputs)
    res = run_bass_kernel_spmd(nc, in_maps, core_ids=list(range(N_CORES)))
    out1 = np.stack([res.results[b]["out01"][0:1, :] for b in range(N_CORES)])
    out2 = np.stack([res.results[b]["out01"][1:2, :] for b in range(N_CORES)])
    return out1.astype(np.float32), out2.astype(np.float32)

